# revision 43
# baseline (speedup 1.0000x reference)
"""Batched conv layer (im2col gather + einsum) as a Bass/Tile TRN2 kernel.

Problem: x (8,16,32,32,64) f32, kernel (8,3,3,64,128) f32
         out[b,i,oh,ow,f] = sum_{kh,kw,c} xpad[b,i,oh+kh-1,ow+kw-1,c] * kernel[b,kh,kw,c,f]
         out (8,16,32,32,128) f32

Sharding: batch dim b across 8 cores (pure data parallel, no collectives).

Per-core device layout (host prepares these):
  xp : (8 pairs, 128, 34*34) f32   partition dim packs 2 images x 64 channels;
                                   free dim is the zero-padded 34x34 image plane
  kd : (128, 9*128) f32            partition dim packs 2 copies of the 64 channels
                                   (one per image in a pair); free dim is
                                   9 taps x 128 output filters
  out: (16, 128, 1024) f32         [image, filter, position]; host transposes back

The conv is computed as 9 shifted matmuls accumulated in PSUM:
  out[f, pos] += ktap[c, f].T @ xwin[c, pos]   for each tap (kh, kw)
Images are processed in pairs occupying PE row-groups 0-63 / 64-127 so two
K=64 matmuls can run concurrently in the 128x128 array.
"""

import os

import numpy as np

import concourse.bass as bass
import concourse.mybir as mybir
from concourse import bacc
from concourse.bass_utils import run_bass_kernel_spmd
from concourse.tile import TileContext

# Static problem config (hardcoded per the harness contract)
B, I, H, W, C, F = 8, 16, 32, 32, 64, 128
KD = 3
HP = H + 2  # padded
WP = W + 2
NPOS = H * W          # 1024 output positions per image
NTILE = 512           # positions per PSUM tile (one bank)
NHALF = NPOS // NTILE  # 2
ROWS_PER_TILE = NTILE // W  # 16 output rows per tile
N_CORES = 8

# matmul input dtype: "f16" (default: ~3e-4 rel err, fastest), "f32r"
# (~1.5e-4), "f32" (exact, 4x slower PE), "bf16"
MM_DTYPE = os.environ.get("CONV_MM_DTYPE", "f16")
# weight (stationary operand) dtype: "" = same as MM_DTYPE
W_DTYPE = os.environ.get("CONV_W_DTYPE", "")

_CACHED_NC = None
LAST_RESULTS = None


def _build_nc():
    nc = bacc.Bacc(trn_type="TRN2")

    mm_dt = {
        "f32": mybir.dt.float32,
        "f32r": mybir.dt.float32r,
        "bf16": mybir.dt.bfloat16,
        "f16": mybir.dt.float16,
    }[MM_DTYPE]
    # For f32r, type the DRAM inputs as float32r end-to-end (same 4-byte fp32
    # layout; the PE just reads fewer mantissa bits) so the BIR verifier sees a
    # consistent fp32r producer chain.  For f16 the host pre-casts the inputs.
    if MM_DTYPE in ("f32r", "f16"):
        in_dt = mm_dt
    else:
        in_dt = mybir.dt.float32

    k_dt = mybir.dt.float16 if W_DTYPE == "f16" else in_dt

    xp = nc.declare_dram_parameter("xp", [I // 2, 128, HP * WP], in_dt, isOutput=False)
    kd = nc.declare_dram_parameter("kd", [128, KD * KD * F], k_dt, isOutput=False)
    out = nc.declare_dram_parameter("out", [I, F, NPOS], mybir.dt.float32, isOutput=True)

    with TileContext(nc) as tc:
        with (
            tc.tile_pool(name="kpool", bufs=1) as kpool,
            tc.tile_pool(name="xpool", bufs=8) as xpool,
            tc.tile_pool(name="opool", bufs=32) as opool,
            tc.tile_pool(name="psum",
                         bufs=7 if os.environ.get("CONV_WARMUP", "1") == "1" else 8,
                         space="PSUM") as psum_pool,
        ):
            # PE warm-up: the HAM clock gate runs the PE at 1.2 GHz until it
            # has seen ~3.4us of sustained activity.  Burn dummy matmuls on a
            # memset tile while the input DMAs are in flight so the real
            # matmuls start at 2.4 GHz.
            # PE warm-up: the HAM clock gate runs the PE at 1.2 GHz until it
            # has seen ~3.4us of sustained activity; ~3us of dummy matmuls
            # during the input-DMA window lets the real stream start at
            # 2.4 GHz.  One single well-formed accumulation group (an earlier
            # variant with 28 start/stop singletons + alternating
            # tile_position failed at runtime).
            if os.environ.get("CONV_WARMUP", "1") == "1":
                wtile = kpool.tile([128, 384], mybir.dt.float16, tag="warm_in")
                nc.gpsimd.memset(wtile[:, :], 0.0)
                wpsum = psum_pool.tile([128, 128], mybir.dt.float32, tag="warm", bufs=1)
                for i in range(28):
                    nc.tensor.matmul(
                        wpsum[:, :],
                        wtile[0:64, 256:384],
                        wtile[0:64, 0:128],
                        start=(i == 0), stop=(i == 27),
                    )

            # Loads go on the Scalar-engine HWDGE queue, stores on the Sync
            # queue — two independent FIFOs so the 8 MB of output stores never
            # serialize behind input loads.
            load_eng = nc.gpsimd if MM_DTYPE == "bf16" else nc.scalar

            x_dt = mybir.dt.bfloat16 if MM_DTYPE == "bf16" else in_dt

            # Pair 0 first, split into two row-halves so the first matmuls
            # only wait on 313 KB.  The kernel tile rides the (otherwise idle)
            # Sync queue concurrently.
            xtiles = []
            xtile0 = xpool.tile([128, HP, WP], x_dt, tag="x")
            load_eng.dma_start(out=xtile0[:, 0:18, :].rearrange("p h w -> p (h w)"),
                               in_=xp[0, :, 0:18 * WP])
            ktile = kpool.tile([128, KD * KD, F],
                               mybir.dt.bfloat16 if MM_DTYPE == "bf16" else k_dt)
            if MM_DTYPE == "bf16":
                nc.gpsimd.dma_start(out=ktile.rearrange("p t f -> p (t f)"), in_=kd[:, :])
                nc.gpsimd.dma_start(out=xtile0[:, 18:HP, :].rearrange("p h w -> p (h w)"),
                                    in_=xp[0, :, 18 * WP:HP * WP])
            else:
                # kernel tile on the (otherwise idle at startup) Sync queue,
                # concurrent with pair-0's load on the Scalar queue.  Tap-0
                # weights (32 KB) ride a separate first DMA so the first
                # matmul's dependency lands ~0.6us earlier than the full
                # 295 KB kernel tile.
                nc.sync.dma_start(out=ktile[:, 0, :], in_=kd[:, 0:F])
                nc.sync.dma_start(
                    out=ktile[:, 1:KD * KD, :].rearrange("p t f -> p (t f)"),
                    in_=kd[:, F:KD * KD * F])
                load_eng.dma_start(out=xtile0[:, 18:HP, :].rearrange("p h w -> p (h w)"),
                                   in_=xp[0, :, 18 * WP:HP * WP])
            xtiles.append(xtile0)

            for pair in range(1, I // 2):
                xt = xpool.tile([128, HP, WP], x_dt, name=f"x_{pair}", tag="x")
                load_eng.dma_start(out=xt.rearrange("p h w -> p (h w)"), in_=xp[pair])
                xtiles.append(xt)

            def emit_mm(psums, xtile, schedule):
                # schedule: list of (half, par, t)
                for half, par, t in schedule:
                    kh, kw = divmod(t, KD)
                    oh0 = half * ROWS_PER_TILE
                    p0 = par * 64
                    lhsT = ktile[p0:p0 + 64, t, :]
                    rhs = xtile[p0:p0 + 64, oh0 + kh:oh0 + kh + ROWS_PER_TILE,
                                kw:kw + W]
                    nc.tensor.matmul(
                        psums[half][par][:, :], lhsT, rhs,
                        start=(t == 0), stop=(t == KD * KD - 1),
                    )

            for pair in range(I // 2):
                xtile = xtiles[pair]
                psums = []
                for half in range(NHALF):
                    row = []
                    for par in range(2):
                        ps = psum_pool.tile([128, NTILE], mybir.dt.float32,
                                            name=f"ps_{pair}_{half}_{par}", tag="ps")
                        row.append(ps)
                    psums.append(row)

                if pair == 0:
                    # half-major: half 0 only needs the first row-split load
                    sched = [(h, par, t) for h in range(NHALF)
                             for t in range(KD * KD) for par in range(2)]
                elif pair == I // 2 - 1:
                    # Last pair: taps 0-5 tap-major, then each chain finishes
                    # its last 3 taps as a trio.  Chains alternate PE
                    # row-groups so consecutive trios still overlap, but the
                    # completions spread ~0.6us apart — the DVE copies (681ns
                    # each) then run under the trailing matmuls instead of
                    # serializing after the final one.
                    sched = [(h, par, t) for t in range(KD * KD - 4)
                             for h in range(NHALF) for par in range(2)]
                    for h, par in ((0, 0), (0, 1), (1, 0), (1, 1)):
                        sched += [(h, par, t) for t in range(KD * KD - 4, KD * KD)]
                else:
                    # Taps 0-6 tap-major over all 4 psum chains (maximizes
                    # independent work in the PE queue so LDWEIGHTS stays
                    # hidden); the final two taps chain-grouped so the 4
                    # chains *finish* staggered and their PSUM->SBUF copies
                    # overlap the remaining matmuls instead of piling up
                    # after the last one.
                    sched = [(h, par, t) for t in range(KD * KD - 2)
                             for h in range(NHALF) for par in range(2)]
                    sched += [(h, par, t) for h in range(NHALF)
                              for par in range(2)
                              for t in (KD * KD - 2, KD * KD - 1)]
                emit_mm(psums, xtile, sched)

                for half in range(NHALF):
                    for par in range(2):
                        i_img = pair * 2 + par
                        otile = opool.tile([128, NTILE], mybir.dt.float32,
                                           name=f"o_{pair}_{half}_{par}", tag="o")
                        nc.vector.tensor_copy(out=otile[:, :], in_=psums[half][par][:, :])
                        nc.sync.dma_start(
                            out=out[i_img, :, half * NTILE:(half + 1) * NTILE],
                            in_=otile[:, :],
                        )
    nc.compile()
    return nc


def _prep_core_inputs(x_b: np.ndarray, k_b: np.ndarray):
    """x_b (16,32,32,64) f32, k_b (3,3,64,128) f32 -> device layouts."""
    np_in = np.float16 if MM_DTYPE == "f16" else np.float32
    xpad = np.zeros((I, HP, WP, C), dtype=np_in)
    xpad[:, 1:H + 1, 1:W + 1, :] = x_b
    # (I, HP, WP, C) -> (I, C, HP, WP) -> (I//2, 2*C, HP*WP)
    xp = np.ascontiguousarray(xpad.transpose(0, 3, 1, 2)).reshape(I // 2, 2 * C, HP * WP)

    kc = k_b.reshape(KD * KD, C, F)                       # (9, 64, 128)
    kdup = np.concatenate([kc, kc], axis=1)               # (9, 128, 128)
    kd = np.ascontiguousarray(kdup.transpose(1, 0, 2)).reshape(128, KD * KD * F)
    if W_DTYPE == "f16" or MM_DTYPE == "f16":
        kd = kd.astype(np.float16)
    return {"xp": xp, "kd": kd}


def kernel(**inputs) -> np.ndarray:
    global _CACHED_NC, LAST_RESULTS
    x = np.asarray(inputs["x"], dtype=np.float32)
    k = np.asarray(inputs["kernel"], dtype=np.float32)

    if _CACHED_NC is None:
        _CACHED_NC = _build_nc()
    nc = _CACHED_NC

    in_maps = [_prep_core_inputs(x[b], k[b]) for b in range(B)]
    res = run_bass_kernel_spmd(nc, in_maps, core_ids=list(range(N_CORES)))
    LAST_RESULTS = res

    outs = []
    for b in range(B):
        o = res.results[b]["out"]                          # (16, 128, 1024)
        o = o.transpose(0, 2, 1).reshape(I, H, W, F)       # (16, 32, 32, 128)
        outs.append(o)
    return np.ascontiguousarray(np.stack(outs, axis=0))
